# revision 32
# baseline (speedup 1.0000x reference)
"""Multi-head causal attention (B=2, T=2048, E=1024, H=16, D=64) on 8 TRN2
NeuronCores, tensor-parallel over heads (2 heads per core), all-bf16.
Measured: ~173.6us / 4.56e-3 rel err (vs 2e-2 gate; session baseline 197.9us).

Architecture per core (host sums the 8 partial y):
  Phase-1 blocks (512-t QKV projections) are WOVEN BETWEEN attention blocks
  so their PE-heavy/ACT-idle matmuls hide under the exp(ACT)-bound attention
  phase: p1_0, chain0, A(0,0), p1_4+chain4, A(1,0), p1_1, A(0,1), p1_5, ...
  (p1_after schedule keeps each p1 block TWO attention blocks ahead of its
  consumer so its qT/kT DVE copies clear the backlogged DVE queue in time).
  Each p1 block: q/k pass first (16 MMs, psum tag 'acc' bufs=3), then the v
  pass into the single 'epi' bank (time-shared with proj-y/vtr via strict
  FIFO slot order -- emission order must match execution order or the tile
  slot rotation deadlocks; the scheduler's simulator catches this).

  Attention per si (128-s tile): BOTH heads' QK matmuls emit back-to-back
  into one [128,1024] psum pair tile (2 adjacent banks): h0 lhsT/rhs on
  partitions 0-63, h1 on 64-127 auto-derive PE row tiles (0,0)/(64,0) and
  run CONCURRENTLY (2nd MM retires ~4ns after the 1st). ONE exp op (ACT,
  scale=E^-0.5, ~1147ns for 1024 cols) covers both heads via a [128,2,512]
  strided AP over the pair; the attention phase is ACT-rate-limited at
  ~1150ns/si, PE ~1130ns/si -- balanced. The diagonal tril mask runs on
  GPSIMD (affine_select zeroing t<s; DVE was the busier engine). AV matmuls
  (K=128, M=65 with the ones-column denominator row) lag TWO si behind so
  they never wait on exp and stream back-to-back at 216ns.

  Epilogue (11 steps via the global step_queue, popped at next blocks'
  sub-boundaries; must_finish forces all but the 8 proj steps): eager l
  copies + av psum->sbuf bf16 casts (release av banks so 'acc' bufs=3
  suffices), pick/backT/bcast (1/l onto t-partitions then broadcast over
  d), 8 proj MMs y=avT^T@wproj -> DVE cast -> DMA. The FINAL epilogue
  pairs proj outputs into [128,1024] wei pair-slots (2 MMs, 1 cast, 1 DMA)
  and its avT mul reads av psum directly (casts skipped).
PSUM banks: acc=3 + wei pairs 2x2=4 + epi=1 = 8.

Infra notes: walrus accepts ONE semaphore wait per instruction
(_split_multi_waits hoists extras); GPSIMD cannot touch PSUM and rejects
TensorTensor (memset/affine_select only -- affine_select [128,2,128] bf16
takes ~425ns, fine off the critical path); DMA only from sync/scalar
(HWDGE) or gpsimd (SWDGE, ~1us first-byte -- keep off latency-critical
paths); non-matmul APs need 32-aligned partition starts; matmul lhsT/rhs
share base partition; ACT must stay single-function (Exp) -- table swaps
cost ~1.3-2.7us (a dummy warm exp after p1_0's DMA issues preloads the
table under the DMA shadow). PE: 1.2GHz cold/2.4GHz warm; ~107ns LDW
between full-row matmuls is NOT hidden (no FWL in this walrus), so
back-to-back 512-col MMs run ~320ns in the QKV phase; runtime init ~7us
+ first DMA ~3us head, ~10us NEFF teardown tail are fixed.
"""
import sys
import types

import numpy as np

B, T, E, H, D = 2, 2048, 1024, 16, 64
N_CORES = 8
HPC = H // N_CORES          # heads per core = 2
BT = B * T                  # 4096
DPC = HPC * D               # 128 head-dims per core
SCALE = 1.0 / float(np.sqrt(E))  # NOTE: reference scales by E**-0.5


def _install_ntff_hook():
    if 'antenv.axon_hooks' in sys.modules:
        return
    try:
        sys.path.insert(0, '/root/.axon_site')
        from trn_agent_boot.trn_boot import _ntff_profile_via_ctypes
        hook = _ntff_profile_via_ctypes('/opt/axon/libaxon_pjrt.so')
        mod = types.ModuleType('antenv.axon_hooks')
        mod.get_axon_ntff_profile_hook = lambda: hook
        mod.set_axon_ntff_profile_hook = lambda h: None
        sys.modules['antenv.axon_hooks'] = mod
    except Exception:
        pass


def _split_multi_waits(nc, mybir):
    """This walrus build rejects >1 sync-wait per instruction. Hoist extra
    waits onto EventSemaphore instructions on the same engine just before."""
    for f in nc.m.functions:
        for bb in f.blocks:
            new_insts = []
            changed = False
            for inst in bb.instructions:
                si = inst.sync_info
                if si is not None and len(si.on_wait) > 1:
                    extra = list(si.on_wait[:-1])
                    keep = si.on_wait[-1]
                    for w in extra:
                        ev = mybir.InstEventSemaphore(
                            name=f"I-{nc.next_id()}", ins=[], outs=[])
                        ev.engine = inst.engine
                        ev.sync_info = mybir.SyncInfo(on_wait=[w], on_update=[])
                        new_insts.append(ev)
                    del si.on_wait[:]
                    si.on_wait.append(keep)
                    changed = True
                new_insts.append(inst)
            if changed:
                bb.instructions = new_insts


def _build_nc():
    import concourse.bass as bass
    import concourse.mybir as mybir
    import concourse.tile as tile
    from concourse.masks import make_identity

    f32 = mybir.dt.float32
    bf16 = mybir.dt.bfloat16
    EXP = mybir.ActivationFunctionType.Exp
    DIV = mybir.AluOpType.divide

    nc = bass.Bass('TRN2', num_devices=N_CORES)
    xt = nc.dram_tensor('xt', [E, BT], bf16, kind='ExternalInput')
    wqkv = nc.dram_tensor('wqkv', [E, 3 * DPC], bf16, kind='ExternalInput')
    wproj = nc.dram_tensor('wproj', [DPC, E], bf16, kind='ExternalInput')
    y = nc.dram_tensor('y', [BT, E], bf16, kind='ExternalOutput')

    NTB1 = BT // 512            # 8 t-blocks in phase 1
    NE = E // 128               # 8 e-tiles
    NTB = T // 512              # 4 t-blocks per batch in phase 2

    with tile.TileContext(nc) as tc:
        with tc.tile_pool(name='consts', bufs=1) as consts, \
             tc.tile_pool(name='big', bufs=1) as big, \
             tc.tile_pool(name='work', bufs=1) as work, \
             tc.tile_pool(name='ps', bufs=1, space='PSUM') as ps:

            # ---- constants ----
            ident_f = consts.tile([128, 128], f32)
            make_identity(nc, ident_f)
            ident = consts.tile([128, 128], bf16)
            nc.vector.tensor_copy(ident[:], ident_f[:])
            # ones33x64 bf16: K=1 broadcast rows for the 1/l bcast; rows
            # 0/32 used so lhsT base partition matches rc_bf's row
            ones33x64 = consts.tile([33, 64], bf16)
            nc.gpsimd.memset(ones33x64[:], 1.0)
            # sel_pick [33, 2] bf16: col h picks row 32h (used to move the
            # softmax denominators from free-dim-t onto t-partitions)
            sel_pick = consts.tile([33, 2], bf16)
            nc.gpsimd.memset(sel_pick[:], 0.0)
            nc.gpsimd.memset(sel_pick[0:1, 0:1], 1.0)
            nc.gpsimd.memset(sel_pick[32:33, 1:2], 1.0)
            warm = consts.tile([1, 128], bf16)
            # persistent l tiles (rows 1-31 stay 1.0; sel_pick ignores them)
            l_sbs = [big.tile([33, 512], bf16, name=f'lsb{k}') for k in range(2)]
            for k in range(2):
                nc.gpsimd.memset(l_sbs[k][:], 1.0)

            # ---- weights (DMAs issued inside phase 1 / after it, so the
            # first matmul doesn't sit behind the whole weight load) ----
            wqkv_sb = [consts.tile([128, 3 * DPC], bf16, name=f'wqkv{k}')
                       for k in range(NE)]
            wproj_sb = consts.tile([DPC, E], bf16)

            # ---- persistent activations ----
            qT_sb = [big.tile([128, 512], bf16, name=f'q{j}')
                     for j in range(NTB1)]
            kT_sb = [big.tile([128, 512], bf16, name=f'k{j}')
                     for j in range(NTB1)]
            # v tiles [s, (sc, h, d+1)] per 512-t block: per head 64 dims +
            # ones column (softmax denominator via row 64 of the AV matmul)
            v_sb = [big.tile([128, 4, 2, 65], bf16, name=f'v{j}')
                    for j in range(NTB1)]
            for j in range(NTB1):
                nc.gpsimd.memset(v_sb[j][:, :, :, 64:65], 1.0)

            # ---- phase 1: one 512-t block of QKV projections, q/k pass
            # first (16 matmuls), then the v pass (8 matmuls into the single
            # 'epi' bank, which it time-shares with proj-y/vtr). The q/k
            # pass leads so a late-freed epi bank never stalls the block ----
            def emit_qkv_matmuls(j):
                ts = j * 512
                q_ps = ps.tile([128, 512], f32, tag='acc', bufs=3)
                k_ps = ps.tile([128, 512], f32, tag='acc', bufs=3)
                xts = []
                engs = ((nc.sync, nc.scalar) if j in (0, 4)
                        else (nc.sync, nc.gpsimd))
                for k in range(NE):
                    xt_t = work.tile([128, 512], bf16, tag='xt', bufs=16)
                    xts.append(xt_t)
                    dma_eng = engs[k % len(engs)]
                    alt_eng = engs[(k + 1) % len(engs)]
                    if j == 0:
                        # stream this e-tile's weights just ahead of its x,
                        # on the other queue so both transfers overlap;
                        # mid-deadline tiles (k=2..4, needed ~12us in) go
                        # via the gpsimd SWDGE queue to unclog the two
                        # HWDGE queues for the latency-critical xt tiles
                        w_eng = nc.gpsimd if k in (2, 3, 4) else alt_eng
                        w_eng.dma_start(
                            out=wqkv_sb[k][:],
                            in_=wqkv[k * 128:(k + 1) * 128, :])
                    dma_eng.dma_start(
                        out=xt_t[:], in_=xt[k * 128:(k + 1) * 128, ts:ts + 512])
                    st, sp = (k == 0), (k == NE - 1)
                    nc.tensor.matmul(q_ps[:], wqkv_sb[k][:, 0:128], xt_t[:],
                                     start=st, stop=sp)
                    nc.tensor.matmul(k_ps[:], wqkv_sb[k][:, 128:256], xt_t[:],
                                     start=st, stop=sp)
                vt_ps = ps.tile([128, 512], f32, tag='epi', bufs=1)
                for k in range(NE):
                    nc.tensor.matmul(vt_ps[:], wqkv_sb[k][:, 256:384], xts[k],
                                     start=(k == 0), stop=(k == NE - 1))
                nc.vector.tensor_copy(qT_sb[j][:], q_ps[:])
                nc.vector.tensor_copy(kT_sb[j][:], k_ps[:])
                return vt_ps

            def make_v_chain(j, vt_ps):
                vt_sb = work.tile([128, 512], bf16, tag='vt', bufs=2)
                state = {}

                def step_a():
                    nc.vector.tensor_copy(vt_sb[:], vt_ps[:])
                    vtr = ps.tile([128, 512], bf16, tag='epi', bufs=1)
                    for sc in range(4):
                        nc.tensor.transpose(vtr[:, sc * 128:(sc + 1) * 128],
                                            vt_sb[:, sc * 128:(sc + 1) * 128],
                                            ident[:])
                    state['vtr'] = vtr

                def step_b():
                    nc.vector.tensor_copy(
                        v_sb[j][:, :, :, 0:64],
                        state['vtr'].rearrange('p (c h e) -> p c h e',
                                               c=4, h=2))

                return [step_a, step_b]

            # ---- attention for one t-block, with block i-1's epilogue
            # interleaved at (si, h) sub-boundaries ----
            step_queue = []

            def emit_attn_block(b, tb, must_finish=0):
                """must_finish: number of queue-front steps that MUST run
                inside this block (frees av psum banks for block i+2);
                the rest pace at ~1 step per 2 sub-boundaries and may spill
                into later blocks via the global queue.

                Both heads' QK matmuls are emitted back-to-back into one
                [128,1024] psum pair tile (2 adjacent banks): h0's lhsT/rhs
                sit on partitions 0-63, h1's on 64-127, so they lower to PE
                row tiles (0,0)/(64,0) and execute concurrently. One exp op
                and one mask op then cover both heads."""
                n_si = 4 * (tb + 1)
                av_pss = [ps.tile([65, 512], f32, tag='acc', bufs=3,
                                  name=f'av{b}_{tb}_{h}')
                          for h in range(HPC)]
                pending = []       # av matmul args, two si behind scores
                popped = 0
                for si in range(n_si):
                    j = b * NTB + si // 4
                    sc = si % 4
                    woff = max(0, (si - 4 * tb) * 128)
                    diag = si >= 4 * tb
                    # pop epilogue steps at sub-boundaries from sub 4 on:
                    # every boundary while the required steps are not yet
                    # done or there is backlog, else every other boundary
                    for k in range(HPC):
                        idx = 2 * si + k - 4
                        if idx >= 0 and step_queue:
                            run = (popped < must_finish
                                   or len(step_queue) > 6
                                   or idx % 2 == 0)
                            if run:
                                step_queue.pop(0)()
                                popped += 1
                    w_pair = ps.tile([128, 1024], f32, tag='wei', bufs=2)
                    wp = w_pair.rearrange('p (h t) -> p h t', h=2)
                    for h in range(HPC):
                        hd = h * 64
                        nc.tensor.matmul(
                            wp[:, h, woff:512],
                            kT_sb[j][hd:hd + 64, sc * 128:(sc + 1) * 128],
                            qT_sb[b * NTB + tb][hd:hd + 64, woff:512],
                            start=True, stop=True)
                    wt = work.tile([128, 2, 512], bf16, tag='weiT', bufs=8)
                    nc.scalar.activation(wt[:, :, woff:512],
                                         wp[:, :, woff:512],
                                         EXP, scale=SCALE)
                    if diag:
                        nc.gpsimd.affine_select(
                            out=wt[:, :, woff:woff + 128],
                            in_=wt[:, :, woff:woff + 128],
                            compare_op=mybir.AluOpType.is_ge,
                            fill=0.0, base=0, pattern=[[0, 2], [1, 128]],
                            channel_multiplier=-1)
                    while len(pending) > HPC:
                        nc.tensor.matmul(**pending.pop(0))
                    pending += [dict(
                        out=av_pss[h][:, woff:512],
                        lhsT=v_sb[j][:, sc, h, :],
                        rhs=wt[:, h, woff:512],
                        start=(si == 0), stop=(si == n_si - 1),
                        skip_group_check=True) for h in range(HPC)]
                for p in pending:
                    nc.tensor.matmul(**p)
                # any required steps not yet popped run now (bunched)
                while popped < must_finish and step_queue:
                    step_queue.pop(0)()
                    popped += 1
                return av_pss

            # ---- epilogue steps for one block: pick + backT + bcast + 8 proj
            def make_epi_steps(b, tb, seq, av_pss, final=False):
                t0 = (b * NTB + tb) * 512
                l_sb = l_sbs[seq % 2]
                # the l copies and the av psum->sbuf casts run eagerly
                # (DVE, right after the last AV): they release the av banks
                # ~1.3us into the next block so 'acc' works with bufs=3;
                # everything else is a step interleaved into the next block
                av_sb = work.tile([128, 512], bf16, tag='avsb', bufs=2,
                                  name=f'avsb{seq}')
                for h in range(HPC):
                    nc.vector.tensor_copy(l_sb[32 * h:32 * h + 1, :],
                                          av_pss[h][64:65, :])
                    if not final:
                        # final block: no successor needs the av banks, so
                        # skip the eager casts; bcast reads psum directly
                        nc.vector.tensor_copy(av_sb[64 * h:64 * h + 64, :],
                                              av_pss[h][0:64, :])
                avT = work.tile([128, 512], bf16, tag='avT', bufs=2,
                                name=f'avT{seq}')
                rc_t = work.tile([128, 4, 2], bf16, tag='rct', bufs=2)
                rc_bf = work.tile([33, 512], bf16, tag='rcb', bufs=2)

                def step_pick():
                    # denominators -> t-partitions; reciprocal over 8 elems
                    # (latency-optimal: used only for the final, un-hideable
                    # epilogue chain)
                    lT_ps = ps.tile([128, 4, 2], f32, tag='wei', bufs=2)
                    for c in range(4):
                        nc.tensor.matmul(lT_ps[:, c, :],
                                         l_sb[:, c * 128:(c + 1) * 128],
                                         sel_pick[:], start=True, stop=True)
                    with nc.allow_low_precision("1/l bf16 within tolerance"):
                        nc.vector.reciprocal(rc_t[:], lT_ps[:])

                def step_backt():
                    rc_ps = ps.tile([33, 512], bf16, tag='wei', bufs=2)
                    for h in range(HPC):
                        for c in range(4):
                            nc.tensor.transpose(
                                rc_ps[32 * h:32 * h + 1,
                                      c * 128:(c + 1) * 128],
                                rc_t[:, c, h:h + 1], ident[:])
                    nc.vector.tensor_copy(rc_bf[:], rc_ps[:])

                def make_recip(c):
                    # mid-run: direct DVE reciprocal on the l rows, 128 cols
                    # per step (~900ns each; iterative divide ~8 cyc/elem).
                    # Zero PE cost -- replaces 12 pick/backT matmuls+LDWs;
                    # rows 1-31 are 1.0 so their reciprocal is harmless
                    def step():
                        with nc.allow_low_precision("1/l bf16 tolerance"):
                            nc.vector.reciprocal(
                                rc_bf[:, c * 128:(c + 1) * 128],
                                l_sb[:, c * 128:(c + 1) * 128])
                    return step

                def step_bcast():
                    # per-head K=1 broadcast: bc[64h:64h+64, t] = 1/l_h[t];
                    # reads only rc_bf rows 0/32 (rows 1-31 are psum garbage)
                    bc_ps = ps.tile([128, 512], f32, tag='wei', bufs=2)
                    for h in range(HPC):
                        nc.tensor.matmul(bc_ps[64 * h:64 * h + 64, :],
                                         ones33x64[32 * h:32 * h + 1, :],
                                         rc_bf[32 * h:32 * h + 1, :],
                                         start=True, stop=True)
                    bc_sb = work.tile([128, 512], bf16, tag='bcs', bufs=2)
                    nc.vector.tensor_copy(bc_sb[:], bc_ps[:])
                    if final:
                        for h in range(HPC):
                            hd = h * 64
                            nc.vector.tensor_mul(avT[hd:hd + 64, :],
                                                 av_pss[h][0:64, :],
                                                 bc_sb[hd:hd + 64, :])
                    else:
                        nc.vector.tensor_mul(avT[:], av_sb[:], bc_sb[:])

                pstate = {}

                def make_proj(tc4, eb):
                    def step():
                        if final:
                            # pair slot: both eb halves in adjacent banks
                            if eb == 0:
                                pstate['yp'] = ps.tile([128, 1024], f32,
                                                       tag='wei', bufs=2,
                                                       name='ypair')
                            y_ps = pstate['yp'][:, eb * 512:(eb + 1) * 512]
                        else:
                            y_ps = ps.tile([128, 512], f32, tag='epi', bufs=1)
                        nc.tensor.matmul(
                            y_ps,
                            avT[:, tc4 * 128:(tc4 + 1) * 128],
                            wproj_sb[:, eb * 512:(eb + 1) * 512],
                            start=True, stop=True)
                        if final:
                            if eb == 0:
                                pstate['ysb'] = work.tile(
                                    [128, 1024], bf16, tag='ysb', bufs=4,
                                    name='ysb')
                            if eb == 1:
                                nc.vector.tensor_copy(pstate['ysb'][:],
                                                      pstate['yp'][:])
                                nc.sync.dma_start(
                                    out=y[t0 + tc4 * 128:
                                          t0 + (tc4 + 1) * 128, :],
                                    in_=pstate['ysb'][:])
                        else:
                            y_sb = work.tile([128, 512], bf16, tag='ysb2',
                                             bufs=8)
                            nc.vector.tensor_copy(y_sb[:], y_ps)
                            nc.sync.dma_start(
                                out=y[t0 + tc4 * 128:t0 + (tc4 + 1) * 128,
                                      eb * 512:(eb + 1) * 512],
                                in_=y_sb[:])
                    return step

                step_queue.append(step_pick)
                step_queue.append(step_backt)
                step_queue.append(step_bcast)
                for tc4 in range(4):
                    for eb in range(2):
                        step_queue.append(make_proj(tc4, eb))

            # ---- emission: phase-1 prologue (j0 with inline v chain, j4
            # with queued chain), then attention blocks with the remaining
            # phase-1 blocks woven between them so the PE-heavy/ACT-idle
            # projection work hides under the ACT-bound attention phase.
            # Each p1 block's v chain is queued at the FRONT of step_queue
            # so it pops during the following attention block, in time for
            # the block after that which first reads its v tiles ----
            # dummy PE warm-up: ~3.5us of matmuls on the ident const (no
            # DMA dependency) run during the runtime-init/DMA-wait window,
            # so the HAM clock gate is already at 2.4GHz when real work
            # arrives; the wei pair slot is immediately recycled
            warm_ps = ps.tile([128, 1024], f32, tag='wei', bufs=2,
                              name='warmps')
            for _ in range(30):
                nc.tensor.matmul(warm_ps[:, 0:128], ident[:], ident[:],
                                 start=True, stop=True)
            vt0 = emit_qkv_matmuls(0)
            # wproj is first needed by the first epilogue (~40us in):
            # gpsimd SWDGE keeps it off the prologue's critical HWDGE queues
            nc.gpsimd.dma_start(out=wproj_sb[:], in_=wproj[:])
            # dummy exp AFTER the DMA issues: forces the ~2.7us ACT table
            # load to run now (under the DMA shadow) without delaying the
            # scalar-queue xt DMAs ahead of it
            nc.scalar.activation(warm[:], ident_f[0:1, :], EXP, scale=0.5)
            for f in make_v_chain(0, vt0):
                f()

            # batches interleaved so no big block's epilogue lands in a
            # tiny tb=0 host mid-run (only at the cold start)
            order = [(b, tb) for tb in range(NTB) for b in range(B)]
            p1_after = {0: 1, 1: 5, 2: 2, 3: 6, 4: 3, 5: 7}
            for seq, (b, tb) in enumerate(order):
                # everything except the previous epilogue's 8 proj steps must
                # run inside this block (frees that epilogue's av psum banks);
                # the last block drains the whole queue so the hostless final
                # flush holds only its own epilogue
                mf = max(0, len(step_queue) - 8)
                if seq == len(order) - 1:
                    mf = len(step_queue)
                av_pss = emit_attn_block(b, tb, must_finish=mf)
                make_epi_steps(b, tb, seq, av_pss,
                               final=(seq == len(order) - 1))
                if seq == 0:
                    # p1_4 + inline v chain between A00 and A10 (A10 reads
                    # v(4) at si=0); must precede p1_1 so the single epi
                    # bank's FIFO slot order matches execution order
                    vt4 = emit_qkv_matmuls(4)
                    for f in make_v_chain(4, vt4):
                        f()
                j = p1_after.get(seq)
                if j is not None:
                    vt_ps = emit_qkv_matmuls(j)
                    step_queue[:0] = make_v_chain(j, vt_ps)
            while step_queue:
                step_queue.pop(0)()

    import concourse.mybir as mybir2
    _split_multi_waits(nc, mybir2)
    return nc


_CACHE = {}


def kernel(x, Wq, Wk, Wv, Wproj, bproj):
    _install_ntff_hook()
    import ml_dtypes
    from concourse.bass_utils import run_bass_kernel_spmd

    bf = ml_dtypes.bfloat16
    x = np.asarray(x, dtype=np.float32)
    Wq = np.asarray(Wq, dtype=np.float32)
    Wk = np.asarray(Wk, dtype=np.float32)
    Wv = np.asarray(Wv, dtype=np.float32)
    Wproj = np.asarray(Wproj, dtype=np.float32)
    bproj = np.asarray(bproj, dtype=np.float32)

    if 'nc' not in _CACHE:
        _CACHE['nc'] = _build_nc()
    nc = _CACHE['nc']

    xT = np.ascontiguousarray(x.reshape(BT, E).T).astype(bf)
    in_maps = []
    for c in range(N_CORES):
        h0 = HPC * c
        wqkv_c = np.concatenate(
            [Wq[h0], Wq[h0 + 1], Wk[h0], Wk[h0 + 1], Wv[h0], Wv[h0 + 1]],
            axis=1)                                         # [E, 384]
        wproj_c = np.ascontiguousarray(Wproj[DPC * c: DPC * (c + 1)])
        in_maps.append({'xt': xT,
                        'wqkv': np.ascontiguousarray(wqkv_c).astype(bf),
                        'wproj': wproj_c.astype(bf)})

    res = run_bass_kernel_spmd(nc, in_maps, list(range(N_CORES)))
    ysum = np.zeros((BT, E), dtype=np.float64)
    for c in range(N_CORES):
        ysum += res.results[c]['y'].astype(np.float64)
    out = (ysum + bproj.astype(np.float64)).astype(np.float32)
    return out.reshape(B, T, E)



# revision 33
# speedup vs baseline: 1.0034x; 1.0034x over previous
"""Multi-head causal attention (B=2, T=2048, E=1024, H=16, D=64) on 8 TRN2
NeuronCores, tensor-parallel over heads (2 heads per core), all-bf16.
Measured: ~172.8us / 4.56e-3 rel err (vs 2e-2 gate; session baseline 197.9us).

Architecture per core (host sums the 8 partial y):
  Phase-1 blocks (512-t QKV projections) are WOVEN BETWEEN attention blocks
  so their PE-heavy/ACT-idle matmuls hide under the exp(ACT)-bound attention
  phase: p1_0, chain0, A(0,0), p1_4+chain4, A(1,0), p1_1, A(0,1), p1_5, ...
  (p1_after schedule keeps each p1 block TWO attention blocks ahead of its
  consumer so its qT/kT DVE copies clear the backlogged DVE queue in time).
  Each p1 block: q/k pass first (16 MMs, psum tag 'acc' bufs=3), then the v
  pass into the single 'epi' bank (time-shared with proj-y/vtr via strict
  FIFO slot order -- emission order must match execution order or the tile
  slot rotation deadlocks; the scheduler's simulator catches this).

  Attention per si (128-s tile): BOTH heads' QK matmuls emit back-to-back
  into one [128,1024] psum pair tile (2 adjacent banks): h0 lhsT/rhs on
  partitions 0-63, h1 on 64-127 auto-derive PE row tiles (0,0)/(64,0) and
  run CONCURRENTLY (2nd MM retires ~4ns after the 1st). ONE exp op (ACT,
  scale=E^-0.5, ~1147ns for 1024 cols) covers both heads via a [128,2,512]
  strided AP over the pair; the attention phase is ACT-rate-limited at
  ~1150ns/si, PE ~1130ns/si -- balanced. The diagonal tril mask runs on
  GPSIMD (affine_select zeroing t<s; DVE was the busier engine). AV matmuls
  (K=128, M=65 with the ones-column denominator row) lag TWO si behind so
  they never wait on exp and stream back-to-back at 216ns.

  Epilogue (11 steps via the global step_queue, popped at next blocks'
  sub-boundaries; must_finish forces all but the 8 proj steps): eager l
  copies + av psum->sbuf bf16 casts (release av banks so 'acc' bufs=3
  suffices), pick/backT/bcast (1/l onto t-partitions then broadcast over
  d), 8 proj MMs y=avT^T@wproj -> DVE cast -> DMA. The FINAL epilogue
  pairs proj outputs into [128,1024] wei pair-slots (2 MMs, 1 cast, 1 DMA)
  and its avT mul reads av psum directly (casts skipped).
PSUM banks: acc=3 + wei pairs 2x2=4 + epi=1 = 8.

Infra notes: walrus accepts ONE semaphore wait per instruction
(_split_multi_waits hoists extras); GPSIMD cannot touch PSUM and rejects
TensorTensor (memset/affine_select only -- affine_select [128,2,128] bf16
takes ~425ns, fine off the critical path); DMA only from sync/scalar
(HWDGE) or gpsimd (SWDGE, ~1us first-byte -- keep off latency-critical
paths); non-matmul APs need 32-aligned partition starts; matmul lhsT/rhs
share base partition; ACT must stay single-function (Exp) -- table swaps
cost ~1.3-2.7us (a dummy warm exp after p1_0's DMA issues preloads the
table under the DMA shadow). PE: 1.2GHz cold/2.4GHz warm; ~107ns LDW
between full-row matmuls is NOT hidden (no FWL in this walrus), so
back-to-back 512-col MMs run ~320ns in the QKV phase; runtime init ~7us
+ first DMA ~3us head, ~10us NEFF teardown tail are fixed.
"""
import sys
import types

import numpy as np

B, T, E, H, D = 2, 2048, 1024, 16, 64
N_CORES = 8
HPC = H // N_CORES          # heads per core = 2
BT = B * T                  # 4096
DPC = HPC * D               # 128 head-dims per core
SCALE = 1.0 / float(np.sqrt(E))  # NOTE: reference scales by E**-0.5


def _install_ntff_hook():
    if 'antenv.axon_hooks' in sys.modules:
        return
    try:
        sys.path.insert(0, '/root/.axon_site')
        from trn_agent_boot.trn_boot import _ntff_profile_via_ctypes
        hook = _ntff_profile_via_ctypes('/opt/axon/libaxon_pjrt.so')
        mod = types.ModuleType('antenv.axon_hooks')
        mod.get_axon_ntff_profile_hook = lambda: hook
        mod.set_axon_ntff_profile_hook = lambda h: None
        sys.modules['antenv.axon_hooks'] = mod
    except Exception:
        pass


def _split_multi_waits(nc, mybir):
    """This walrus build rejects >1 sync-wait per instruction. Hoist extra
    waits onto EventSemaphore instructions on the same engine just before."""
    for f in nc.m.functions:
        for bb in f.blocks:
            new_insts = []
            changed = False
            for inst in bb.instructions:
                si = inst.sync_info
                if si is not None and len(si.on_wait) > 1:
                    extra = list(si.on_wait[:-1])
                    keep = si.on_wait[-1]
                    for w in extra:
                        ev = mybir.InstEventSemaphore(
                            name=f"I-{nc.next_id()}", ins=[], outs=[])
                        ev.engine = inst.engine
                        ev.sync_info = mybir.SyncInfo(on_wait=[w], on_update=[])
                        new_insts.append(ev)
                    del si.on_wait[:]
                    si.on_wait.append(keep)
                    changed = True
                new_insts.append(inst)
            if changed:
                bb.instructions = new_insts


def _build_nc():
    import concourse.bass as bass
    import concourse.mybir as mybir
    import concourse.tile as tile
    from concourse.masks import make_identity

    f32 = mybir.dt.float32
    bf16 = mybir.dt.bfloat16
    EXP = mybir.ActivationFunctionType.Exp
    DIV = mybir.AluOpType.divide

    nc = bass.Bass('TRN2', num_devices=N_CORES)
    xt = nc.dram_tensor('xt', [E, BT], bf16, kind='ExternalInput')
    wqkv = nc.dram_tensor('wqkv', [E, 3 * DPC], bf16, kind='ExternalInput')
    wproj = nc.dram_tensor('wproj', [DPC, E], bf16, kind='ExternalInput')
    y = nc.dram_tensor('y', [BT, E], bf16, kind='ExternalOutput')

    NTB1 = BT // 512            # 8 t-blocks in phase 1
    NE = E // 128               # 8 e-tiles
    NTB = T // 512              # 4 t-blocks per batch in phase 2

    with tile.TileContext(nc) as tc:
        with tc.tile_pool(name='consts', bufs=1) as consts, \
             tc.tile_pool(name='big', bufs=1) as big, \
             tc.tile_pool(name='work', bufs=1) as work, \
             tc.tile_pool(name='ps', bufs=1, space='PSUM') as ps:

            # ---- constants ----
            ident_f = consts.tile([128, 128], f32)
            make_identity(nc, ident_f)
            ident = consts.tile([128, 128], bf16)
            nc.vector.tensor_copy(ident[:], ident_f[:])
            # ones33x64 bf16: K=1 broadcast rows for the 1/l bcast; rows
            # 0/32 used so lhsT base partition matches rc_bf's row
            ones33x64 = consts.tile([33, 64], bf16)
            nc.gpsimd.memset(ones33x64[:], 1.0)
            # sel_pick [33, 2] bf16: col h picks row 32h (used to move the
            # softmax denominators from free-dim-t onto t-partitions)
            sel_pick = consts.tile([33, 2], bf16)
            nc.gpsimd.memset(sel_pick[:], 0.0)
            nc.gpsimd.memset(sel_pick[0:1, 0:1], 1.0)
            nc.gpsimd.memset(sel_pick[32:33, 1:2], 1.0)
            warm = consts.tile([1, 128], bf16)
            # persistent l tiles (rows 1-31 stay 1.0; sel_pick ignores them)
            l_sbs = [big.tile([33, 512], bf16, name=f'lsb{k}') for k in range(2)]
            for k in range(2):
                nc.gpsimd.memset(l_sbs[k][:], 1.0)

            # ---- weights (DMAs issued inside phase 1 / after it, so the
            # first matmul doesn't sit behind the whole weight load) ----
            wqkv_sb = [consts.tile([128, 3 * DPC], bf16, name=f'wqkv{k}')
                       for k in range(NE)]
            wproj_sb = consts.tile([DPC, E], bf16)

            # ---- persistent activations ----
            qT_sb = [big.tile([128, 512], bf16, name=f'q{j}')
                     for j in range(NTB1)]
            kT_sb = [big.tile([128, 512], bf16, name=f'k{j}')
                     for j in range(NTB1)]
            # v tiles [s, (sc, h, d+1)] per 512-t block: per head 64 dims +
            # ones column (softmax denominator via row 64 of the AV matmul)
            v_sb = [big.tile([128, 4, 2, 65], bf16, name=f'v{j}')
                    for j in range(NTB1)]
            for j in range(NTB1):
                nc.gpsimd.memset(v_sb[j][:, :, :, 64:65], 1.0)

            # ---- phase 1: one 512-t block of QKV projections, q/k pass
            # first (16 matmuls), then the v pass (8 matmuls into the single
            # 'epi' bank, which it time-shares with proj-y/vtr). The q/k
            # pass leads so a late-freed epi bank never stalls the block ----
            def emit_qkv_matmuls(j):
                ts = j * 512
                q_ps = ps.tile([128, 512], f32, tag='acc', bufs=3)
                k_ps = ps.tile([128, 512], f32, tag='acc', bufs=3)
                xts = []
                engs = ((nc.sync, nc.scalar) if j in (0, 4)
                        else (nc.sync, nc.gpsimd))
                for k in range(NE):
                    xt_t = work.tile([128, 512], bf16, tag='xt', bufs=16)
                    xts.append(xt_t)
                    dma_eng = engs[k % len(engs)]
                    alt_eng = engs[(k + 1) % len(engs)]
                    if j == 0:
                        # stream this e-tile's weights just ahead of its x,
                        # on the other queue so both transfers overlap;
                        # mid-deadline tiles (k=2..4, needed ~12us in) go
                        # via the gpsimd SWDGE queue to unclog the two
                        # HWDGE queues for the latency-critical xt tiles
                        w_eng = nc.gpsimd if k in (2, 3, 4) else alt_eng
                        w_eng.dma_start(
                            out=wqkv_sb[k][:],
                            in_=wqkv[k * 128:(k + 1) * 128, :])
                    dma_eng.dma_start(
                        out=xt_t[:], in_=xt[k * 128:(k + 1) * 128, ts:ts + 512])
                    st, sp = (k == 0), (k == NE - 1)
                    nc.tensor.matmul(q_ps[:], wqkv_sb[k][:, 0:128], xt_t[:],
                                     start=st, stop=sp)
                    nc.tensor.matmul(k_ps[:], wqkv_sb[k][:, 128:256], xt_t[:],
                                     start=st, stop=sp)
                vt_ps = ps.tile([128, 512], f32, tag='epi', bufs=1)
                for k in range(NE):
                    nc.tensor.matmul(vt_ps[:], wqkv_sb[k][:, 256:384], xts[k],
                                     start=(k == 0), stop=(k == NE - 1))
                nc.vector.tensor_copy(qT_sb[j][:], q_ps[:])
                nc.vector.tensor_copy(kT_sb[j][:], k_ps[:])
                return vt_ps

            def make_v_chain(j, vt_ps):
                vt_sb = work.tile([128, 512], bf16, tag='vt', bufs=2)
                state = {}

                def step_a():
                    nc.vector.tensor_copy(vt_sb[:], vt_ps[:])
                    vtr = ps.tile([128, 512], bf16, tag='epi', bufs=1)
                    for sc in range(4):
                        nc.tensor.transpose(vtr[:, sc * 128:(sc + 1) * 128],
                                            vt_sb[:, sc * 128:(sc + 1) * 128],
                                            ident[:])
                    state['vtr'] = vtr

                def step_b():
                    nc.vector.tensor_copy(
                        v_sb[j][:, :, :, 0:64],
                        state['vtr'].rearrange('p (c h e) -> p c h e',
                                               c=4, h=2))

                return [step_a, step_b]

            # ---- attention for one t-block, with block i-1's epilogue
            # interleaved at (si, h) sub-boundaries ----
            step_queue = []

            def emit_attn_block(b, tb, must_finish=0):
                """must_finish: number of queue-front steps that MUST run
                inside this block (frees av psum banks for block i+2);
                the rest pace at ~1 step per 2 sub-boundaries and may spill
                into later blocks via the global queue.

                Both heads' QK matmuls are emitted back-to-back into one
                [128,1024] psum pair tile (2 adjacent banks): h0's lhsT/rhs
                sit on partitions 0-63, h1's on 64-127, so they lower to PE
                row tiles (0,0)/(64,0) and execute concurrently. One exp op
                and one mask op then cover both heads."""
                n_si = 4 * (tb + 1)
                av_pss = [ps.tile([65, 512], f32, tag='acc', bufs=3,
                                  name=f'av{b}_{tb}_{h}')
                          for h in range(HPC)]
                pending = []       # av matmul args, two si behind scores
                popped = 0
                for si in range(n_si):
                    j = b * NTB + si // 4
                    sc = si % 4
                    woff = max(0, (si - 4 * tb) * 128)
                    diag = si >= 4 * tb
                    # pop epilogue steps at sub-boundaries from sub 4 on:
                    # every boundary while the required steps are not yet
                    # done or there is backlog, else every other boundary
                    for k in range(HPC):
                        idx = 2 * si + k - 4
                        if idx >= 0 and step_queue:
                            run = (popped < must_finish
                                   or len(step_queue) > 6
                                   or idx % 2 == 0)
                            if run:
                                step_queue.pop(0)()
                                popped += 1
                    w_pair = ps.tile([128, 1024], f32, tag='wei', bufs=2)
                    wp = w_pair.rearrange('p (h t) -> p h t', h=2)
                    for h in range(HPC):
                        hd = h * 64
                        nc.tensor.matmul(
                            wp[:, h, woff:512],
                            kT_sb[j][hd:hd + 64, sc * 128:(sc + 1) * 128],
                            qT_sb[b * NTB + tb][hd:hd + 64, woff:512],
                            start=True, stop=True)
                    wt = work.tile([128, 2, 512], bf16, tag='weiT', bufs=8)
                    nc.scalar.activation(wt[:, :, woff:512],
                                         wp[:, :, woff:512],
                                         EXP, scale=SCALE)
                    if diag:
                        nc.gpsimd.affine_select(
                            out=wt[:, :, woff:woff + 128],
                            in_=wt[:, :, woff:woff + 128],
                            compare_op=mybir.AluOpType.is_ge,
                            fill=0.0, base=0, pattern=[[0, 2], [1, 128]],
                            channel_multiplier=-1)
                    while len(pending) > HPC:
                        nc.tensor.matmul(**pending.pop(0))
                    pending += [dict(
                        out=av_pss[h][:, woff:512],
                        lhsT=v_sb[j][:, sc, h, :],
                        rhs=wt[:, h, woff:512],
                        start=(si == 0), stop=(si == n_si - 1),
                        skip_group_check=True) for h in range(HPC)]
                for p in pending:
                    nc.tensor.matmul(**p)
                # any required steps not yet popped run now (bunched)
                while popped < must_finish and step_queue:
                    step_queue.pop(0)()
                    popped += 1
                return av_pss

            # ---- epilogue steps for one block: pick + backT + bcast + 8 proj
            def make_epi_steps(b, tb, seq, av_pss, final=False):
                t0 = (b * NTB + tb) * 512
                l_sb = l_sbs[seq % 2]
                # the l copies and the av psum->sbuf casts run eagerly
                # (DVE, right after the last AV): they release the av banks
                # ~1.3us into the next block so 'acc' works with bufs=3;
                # everything else is a step interleaved into the next block
                av_sb = work.tile([128, 512], bf16, tag='avsb', bufs=2,
                                  name=f'avsb{seq}')
                for h in range(HPC):
                    nc.vector.tensor_copy(l_sb[32 * h:32 * h + 1, :],
                                          av_pss[h][64:65, :])
                    if not final:
                        # final block: no successor needs the av banks, so
                        # skip the eager casts; bcast reads psum directly
                        nc.vector.tensor_copy(av_sb[64 * h:64 * h + 64, :],
                                              av_pss[h][0:64, :])
                avT = work.tile([128, 512], bf16, tag='avT', bufs=2,
                                name=f'avT{seq}')
                rc_t = work.tile([128, 4, 2], bf16, tag='rct', bufs=2)
                rc_bf = work.tile([33, 512], bf16, tag='rcb', bufs=2)

                def step_pick():
                    # denominators -> t-partitions; reciprocal over 8 elems
                    # (latency-optimal: used only for the final, un-hideable
                    # epilogue chain)
                    lT_ps = ps.tile([128, 4, 2], f32, tag='wei', bufs=2)
                    for c in range(4):
                        nc.tensor.matmul(lT_ps[:, c, :],
                                         l_sb[:, c * 128:(c + 1) * 128],
                                         sel_pick[:], start=True, stop=True)
                    with nc.allow_low_precision("1/l bf16 within tolerance"):
                        nc.vector.reciprocal(rc_t[:], lT_ps[:])

                def step_backt():
                    rc_ps = ps.tile([33, 512], bf16, tag='wei', bufs=2)
                    for h in range(HPC):
                        for c in range(4):
                            nc.tensor.transpose(
                                rc_ps[32 * h:32 * h + 1,
                                      c * 128:(c + 1) * 128],
                                rc_t[:, c, h:h + 1], ident[:])
                    nc.vector.tensor_copy(rc_bf[:], rc_ps[:])

                def make_recip(c):
                    # mid-run: direct DVE reciprocal on the l rows, 128 cols
                    # per step (~900ns each; iterative divide ~8 cyc/elem).
                    # Zero PE cost -- replaces 12 pick/backT matmuls+LDWs;
                    # rows 1-31 are 1.0 so their reciprocal is harmless
                    def step():
                        with nc.allow_low_precision("1/l bf16 tolerance"):
                            nc.vector.reciprocal(
                                rc_bf[:, c * 128:(c + 1) * 128],
                                l_sb[:, c * 128:(c + 1) * 128])
                    return step

                def step_bcast():
                    # per-head K=1 broadcast: bc[64h:64h+64, t] = 1/l_h[t];
                    # reads only rc_bf rows 0/32 (rows 1-31 are psum garbage)
                    bc_ps = ps.tile([128, 512], f32, tag='wei', bufs=2)
                    for h in range(HPC):
                        nc.tensor.matmul(bc_ps[64 * h:64 * h + 64, :],
                                         ones33x64[32 * h:32 * h + 1, :],
                                         rc_bf[32 * h:32 * h + 1, :],
                                         start=True, stop=True)
                    bc_sb = work.tile([128, 512], bf16, tag='bcs', bufs=2)
                    nc.vector.tensor_copy(bc_sb[:], bc_ps[:])
                    if final:
                        for h in range(HPC):
                            hd = h * 64
                            nc.vector.tensor_mul(avT[hd:hd + 64, :],
                                                 av_pss[h][0:64, :],
                                                 bc_sb[hd:hd + 64, :])
                    else:
                        nc.vector.tensor_mul(avT[:], av_sb[:], bc_sb[:])

                pstate = {}

                def make_proj(tc4, eb):
                    def step():
                        if final:
                            # pair slot: both eb halves in adjacent banks
                            if eb == 0:
                                pstate['yp'] = ps.tile([128, 1024], f32,
                                                       tag='wei', bufs=2,
                                                       name='ypair')
                            y_ps = pstate['yp'][:, eb * 512:(eb + 1) * 512]
                        else:
                            y_ps = ps.tile([128, 512], f32, tag='epi', bufs=1)
                        nc.tensor.matmul(
                            y_ps,
                            avT[:, tc4 * 128:(tc4 + 1) * 128],
                            wproj_sb[:, eb * 512:(eb + 1) * 512],
                            start=True, stop=True)
                        if final:
                            if eb == 0:
                                pstate['ysb'] = work.tile(
                                    [128, 1024], bf16, tag='ysb', bufs=4,
                                    name='ysb')
                            if eb == 1:
                                nc.vector.tensor_copy(pstate['ysb'][:],
                                                      pstate['yp'][:])
                                nc.sync.dma_start(
                                    out=y[t0 + tc4 * 128:
                                          t0 + (tc4 + 1) * 128, :],
                                    in_=pstate['ysb'][:])
                        else:
                            y_sb = work.tile([128, 512], bf16, tag='ysb2',
                                             bufs=8)
                            nc.vector.tensor_copy(y_sb[:], y_ps)
                            nc.sync.dma_start(
                                out=y[t0 + tc4 * 128:t0 + (tc4 + 1) * 128,
                                      eb * 512:(eb + 1) * 512],
                                in_=y_sb[:])
                    return step

                step_queue.append(step_pick)
                step_queue.append(step_backt)
                step_queue.append(step_bcast)
                for tc4 in range(4):
                    for eb in range(2):
                        step_queue.append(make_proj(tc4, eb))

            # ---- emission: phase-1 prologue (j0 with inline v chain, j4
            # with queued chain), then attention blocks with the remaining
            # phase-1 blocks woven between them so the PE-heavy/ACT-idle
            # projection work hides under the ACT-bound attention phase.
            # Each p1 block's v chain is queued at the FRONT of step_queue
            # so it pops during the following attention block, in time for
            # the block after that which first reads its v tiles ----
            # dummy PE warm-up: ~3.5us of matmuls on the ident const (no
            # DMA dependency) run during the runtime-init/DMA-wait window,
            # so the HAM clock gate is already at 2.4GHz when real work
            # arrives; the wei pair slot is immediately recycled
            warm_ps = ps.tile([128, 1024], f32, tag='wei', bufs=2,
                              name='warmps')
            for _ in range(30):
                nc.tensor.matmul(warm_ps[:, 0:128], ident[:], ident[:],
                                 start=True, stop=True)
            vt0 = emit_qkv_matmuls(0)
            # wproj is first needed by the first epilogue
            nc.sync.dma_start(out=wproj_sb[:], in_=wproj[:])
            # dummy exp AFTER the DMA issues: forces the ~2.7us ACT table
            # load to run now (under the DMA shadow) without delaying the
            # scalar-queue xt DMAs ahead of it
            nc.scalar.activation(warm[:], ident_f[0:1, :], EXP, scale=0.5)
            for f in make_v_chain(0, vt0):
                f()

            # batches interleaved so no big block's epilogue lands in a
            # tiny tb=0 host mid-run (only at the cold start)
            order = [(b, tb) for tb in range(NTB) for b in range(B)]
            p1_after = {0: 1, 1: 5, 2: 2, 3: 6, 4: 3, 5: 7}
            for seq, (b, tb) in enumerate(order):
                # everything except the previous epilogue's 8 proj steps must
                # run inside this block (frees that epilogue's av psum banks);
                # the last block drains the whole queue so the hostless final
                # flush holds only its own epilogue
                mf = max(0, len(step_queue) - 8)
                if seq == len(order) - 1:
                    mf = len(step_queue)
                av_pss = emit_attn_block(b, tb, must_finish=mf)
                make_epi_steps(b, tb, seq, av_pss,
                               final=(seq == len(order) - 1))
                if seq == 0:
                    # p1_4 + inline v chain between A00 and A10 (A10 reads
                    # v(4) at si=0); must precede p1_1 so the single epi
                    # bank's FIFO slot order matches execution order
                    vt4 = emit_qkv_matmuls(4)
                    for f in make_v_chain(4, vt4):
                        f()
                j = p1_after.get(seq)
                if j is not None:
                    vt_ps = emit_qkv_matmuls(j)
                    step_queue[:0] = make_v_chain(j, vt_ps)
            while step_queue:
                step_queue.pop(0)()

    import concourse.mybir as mybir2
    _split_multi_waits(nc, mybir2)
    return nc


_CACHE = {}


def kernel(x, Wq, Wk, Wv, Wproj, bproj):
    _install_ntff_hook()
    import ml_dtypes
    from concourse.bass_utils import run_bass_kernel_spmd

    bf = ml_dtypes.bfloat16
    x = np.asarray(x, dtype=np.float32)
    Wq = np.asarray(Wq, dtype=np.float32)
    Wk = np.asarray(Wk, dtype=np.float32)
    Wv = np.asarray(Wv, dtype=np.float32)
    Wproj = np.asarray(Wproj, dtype=np.float32)
    bproj = np.asarray(bproj, dtype=np.float32)

    if 'nc' not in _CACHE:
        _CACHE['nc'] = _build_nc()
    nc = _CACHE['nc']

    xT = np.ascontiguousarray(x.reshape(BT, E).T).astype(bf)
    in_maps = []
    for c in range(N_CORES):
        h0 = HPC * c
        wqkv_c = np.concatenate(
            [Wq[h0], Wq[h0 + 1], Wk[h0], Wk[h0 + 1], Wv[h0], Wv[h0 + 1]],
            axis=1)                                         # [E, 384]
        wproj_c = np.ascontiguousarray(Wproj[DPC * c: DPC * (c + 1)])
        in_maps.append({'xt': xT,
                        'wqkv': np.ascontiguousarray(wqkv_c).astype(bf),
                        'wproj': wproj_c.astype(bf)})

    res = run_bass_kernel_spmd(nc, in_maps, list(range(N_CORES)))
    ysum = np.zeros((BT, E), dtype=np.float64)
    for c in range(N_CORES):
        ysum += res.results[c]['y'].astype(np.float64)
    out = (ysum + bproj.astype(np.float64)).astype(np.float32)
    return out.reshape(B, T, E)

